# revision 81
# baseline (speedup 1.0000x reference)
"""Trainium2 Bass kernel for CustomMultiHeadAttention (B=4, S=1024, D=1024, H=16, Dh=64).

Sharding: 8 cores = (batch b in 0..3) x (head-group g in 0..1).
Core (b, g) computes heads 8g..8g+7 for ALL 1024 positions of batch b:
  - Q/K/V projections use only the 512 dout columns of Wq/Wk/Wv for its heads
  - attention (causal softmax) for its 8 heads over the full sequence
  - a PARTIAL output projection y_part = ctx_g @ Wo[512g:512(g+1), :]
The host sums the two partial outputs per batch (free for HW time).

Vs batch x parity sharding this halves every projection's per-core work
(no K/V duplication), halves weight DMA (4MB vs 8MB), and keeps the
causal mask a single constant lower-tri block.

Pipeline (transposed layout, PE-centric):
  KT = rope(Wk^T x^T), QT = rope(Wq^T x^T)  - rope via perm-matmul + DVE
  V in natural [s, dout] 65-wide slots [V(64) | ones(1)] per head
  scores sc[kv, q] = KT_h^T QT_h per 128-kv block j, q processed in two
  512-col halves; exp on ScalarE (scale=1/8); causal mask = tri multiply
  on the diagonal block; ctx accumulates with lhsT=[V|1] so psum row 64
  is the softmax denominator (free); normalize via reciprocal + PE
  broadcast; y_part = ctx^T Wo_half (natural layout, DMA out).
"""

import threading

import numpy as np

B, S, D, H, Dh = 4, 1024, 1024, 16, 64
P = 128
N_CORES = 8
NT = D // P    # 8 k-tiles along din
TT = 4         # dout-half tiles (512 / 128)
DG = 512       # dout per head group
VS = 65        # V slot width: [V(64) | ones(1)] per head

_cache = {}
_lock = threading.Lock()


def _build_program(taps=False):
    import concourse.bass as bass  # noqa: F401
    import concourse.mybir as mybir
    import concourse.tile as tile
    from concourse import bacc

    dt = mybir.dt
    f16, f32 = dt.float16, dt.float32
    AF = mybir.ActivationFunctionType

    nc = bacc.Bacc("TRN2", target_bir_lowering=False, debug=False,
                   num_devices=N_CORES)

    def ein(name, shape):
        return nc.dram_tensor(name, shape, f16, kind="ExternalInput").ap()

    xt_sh = ein("xt_sh", [NT, P, S])      # x[b]^T, host-transposed
                                          # (k-tile major: contiguous DMA)
    wq_e = ein("wq", [D, DG])             # Wq[:, 512g:512(g+1)]
    wk_e = ein("wk", [D, DG])
    wv_e = ein("wv", [D, DG])
    wo_e = ein("wo", [DG, D])             # Wo[512g:512(g+1), :]
    bqt_e = nc.dram_tensor("bqt", [P, TT], f32, kind="ExternalInput").ap()
    bkt_e = nc.dram_tensor("bkt", [P, TT], f32, kind="ExternalInput").ap()
    bv_e = ein("bv", [1, DG])
    cosk_e = ein("cosk", [P, S])
    sink_e = ein("sink", [P, S])
    tri2_e = ein("tri2", [P, 2, P])       # causal mask, replicated x2
    p128_e = ein("p128", [P, P])
    i64_e = ein("i64", [64, 64])
    y_sh = nc.dram_tensor("y_sh", [S, D], f16, kind="ExternalOutput").ap()
    tap_ext = {}
    if taps:
        for tn, shape in (("qz", [P, TT, 2, S]), ("kt", [P, TT, S]),
                          ("v1", [P, NT, 8 * VS]), ("cn", [P, TT, S])):
            tap_ext[tn] = nc.dram_tensor("dbg_" + tn, shape, f16,
                                         kind="ExternalOutput").ap()

    with tile.TileContext(nc) as tc:
        from contextlib import ExitStack
        with ExitStack() as ctx:
            big = ctx.enter_context(tc.tile_pool(name="big", bufs=1))

            xT = big.tile([P, NT, S], f16, tag="xT")       # x[b]^T [din, s]
            wq = big.tile([P, NT, DG], f16, tag="wq")
            wk = big.tile([P, NT, DG], f16, tag="wk")
            wv = big.tile([P, NT, DG], f16, tag="wv")
            wo = big.tile([P, TT, D], f16, tag="wo")
            bqt = big.tile([P, TT], f32, tag="bqt")
            bkt = big.tile([P, TT], f32, tag="bkt")
            bv_sb = big.tile([1, DG], f16, tag="bv")
            # rope'd Q^T, per-head zero-padded: qz[0:64, p, 0] = head 2p,
            # qz[64:128, p, 1] = head 2p+1, other halves zero. Scores use
            # the full-128-row kt tile as a SHARED lhsT for both heads;
            # the zero half of qz kills the other head's contribution.
            # (Keeps every attention matmul in plain 128-row mode: the
            # 64-row T8-tiled scores + 65-wide ctx combination is fatal
            # on HW.)
            qz = big.tile([P, TT, 2, S], f16, tag="qz")
            kt = big.tile([P, TT, S], f16, tag="kt")       # rope'd K^T
            v1 = big.tile([P, NT, 8 * VS], f16, tag="v1")  # [V|1] slots
            cn = big.tile([P, TT, S], f16, tag="cn")       # normalized ctx^T
            cosk = big.tile([P, S], f16, tag="cosk")
            sink = big.tile([P, S], f16, tag="sink")
            tri2 = big.tile([P, 2, P], f16, tag="tri2")
            p128 = big.tile([P, P], f16, tag="p128")
            i64 = big.tile([64, 64], f16, tag="i64")
            ones = big.tile([P, DG], f16, tag="ones")
            warm = big.tile([1, 16], f16, tag="warm")

            # ---- input DMAs ----
            # the K-proj critical path needs (xT[k], wk[k]) pairs as
            # early as possible; give xT the sync queue to itself and
            # wk the scalar queue so the two streams run in parallel.
            for k in range(NT):
                nc.sync.dma_start(xT[:, k, :], xt_sh[k])
                nc.scalar.dma_start(wk[:, k, :],
                                    wk_e[P * k:P * (k + 1), :])
                if k == 0:
                    for t, e in ((p128, p128_e), (bkt, bkt_e)):
                        nc.gpsimd.dma_start(t[:], e[:])
                if k == 1:
                    for t, e in ((cosk, cosk_e), (sink, sink_e)):
                        nc.gpsimd.dma_start(t[:], e[:])
            for k in range(NT):
                nc.gpsimd.dma_start(wv[:, k, :],
                                    wv_e[P * k:P * (k + 1), :])
            nc.gpsimd.dma_start(bv_sb[:], bv_e[:])
            for k in range(NT):
                nc.sync.dma_start(wq[:, k, :], wq_e[P * k:P * (k + 1), :])
                if k == 0:
                    nc.sync.dma_start(bqt[:], bqt_e[:])
            nc.scalar.dma_start(tri2[:], tri2_e[:])
            nc.scalar.dma_start(i64[:], i64_e[:])
            for t in range(TT):
                nc.scalar.dma_start(wo[:, t, :],
                                    wo_e[P * t:P * (t + 1), :])

            nc.vector.memset(qz[:], 0.0)
            nc.any.memset(ones[:], 1.0)
            v1r = v1.rearrange("p t (h c) -> p t h c", c=VS)
            for t in range(NT):
                nc.any.memset(v1r[:, t, :, 64:65], 1.0)
            # preload the exp table on ScalarE so the first real exp
            # doesn't pay ACT_TABLE_LOAD on the critical path
            nc.scalar.activation(warm[:], ones[0:1, 0:16], AF.Exp, scale=0.01)

            # ---- projections + rope + attention, phased pools ----
            ev = ctx.enter_context(tc.tile_pool(name="ev", bufs=3))
            npl = ctx.enter_context(tc.tile_pool(name="npl", bufs=2))

            # rope is emitted in two stages with a 1-chunk software
            # pipeline: the perm matmul of chunk c is issued after chunk
            # c+1's k-chain so the in-order PE queue never waits on the
            # DVE evac of chunk c.
            rope_pend = []

            def rope_finish(pp):
                if not rope_pend:
                    return
                raw, dsts, csl = rope_pend.pop(0)
                pq = pp.tile([P, DG], f32, tag="ps", name="pq")
                nc.tensor.matmul(pq[:], p128[:], raw[:],
                                 start=True, stop=True)
                t1 = ev.tile([P, DG], f16, tag="t1", name="t1")
                nc.vector.tensor_mul(t1[:], raw[:], cosk[:, csl])
                t2 = ev.tile([P, DG], f16, tag="t2", name="t2")
                nc.vector.tensor_mul(t2[:], pq[:], sink[:, csl])
                for rs, dst in dsts:
                    nc.vector.tensor_add(dst, t1[rs, :], t2[rs, :])

            norm_pend = []

            def rope_evac(ps, dsts, bias, csl):
                # psum evac with fused per-partition bias add
                raw = ev.tile([P, DG], f16, tag="raw", name="raw",
                              bufs=12)
                nc.vector.tensor_scalar_add(raw[:], ps[:], bias)
                rope_pend.append((raw, dsts, csl))

            def proj_k_group(pp, ts):
                # k-major accumulation over 2 dout tiles x 2 s-chunks so
                # the chains start as soon as the first (xT, wk) DMA
                # pair lands instead of waiting for all of wk
                chunks = [(t, n2) for t in ts for n2 in range(2)]
                cps = {c: pp.tile([P, DG], f32, tag="ps",
                                  name=f"kp{c[0]}{c[1]}") for c in chunks}
                for k in range(NT):
                    for (t, n2) in chunks:
                        nc.tensor.matmul(cps[(t, n2)][:],
                                         wk[:, k, P * t:P * (t + 1)],
                                         xT[:, k,
                                            DG * n2:DG * (n2 + 1)],
                                         start=(k == 0),
                                         stop=(k == NT - 1))
                return [(cps[(t, n2)],
                         [(slice(0, P), kt[:, t,
                                          DG * n2:DG * (n2 + 1)])],
                         bkt[:, t:t + 1],
                         slice(DG * n2, DG * (n2 + 1)))
                        for (t, n2) in chunks]

            def proj_q(pp, t):
                wsl = slice(P * t, P * (t + 1))
                for n in range(2):
                    csl = slice(DG * n, DG * (n + 1))
                    ps = pp.tile([P, DG], f32, tag="ps", name="qp")
                    for k in range(NT):
                        nc.tensor.matmul(ps[:], wq[:, k, wsl],
                                         xT[:, k, csl],
                                         start=(k == 0),
                                         stop=(k == NT - 1))
                    rope_evac(ps,
                              [(slice(0, 64), qz[0:64, t, 0, csl]),
                               (slice(64, P), qz[64:P, t, 1, csl])],
                              bqt[:, t:t + 1], csl)
                    rope_finish(pp)

            def proj_v(pp, i):
                # V s-block i: natural [s, dout] into 65-wide slots;
                # evac on ScalarE (idle during proj) to keep DVE free
                ssl = slice(P * i, P * (i + 1))
                vp = pp.tile([P, DG], f32, tag="ps", name="vp")
                for k in range(NT):
                    nc.tensor.matmul(vp[:], xT[:, k, ssl], wv[:, k, :],
                                     start=(k == 0), stop=False)
                nc.tensor.matmul(vp[:], ones[0:1, 0:P], bv_sb[0:1, :],
                                 start=False, stop=True)
                nc.scalar.activation(
                    v1r[:, i, :, 0:64],
                    vp.rearrange("p (h c) -> p h c", c=64), AF.Copy)

            def attn_pair(sc, cx, p):
                # heads h0 = 2p, h1 = 2p+1. Scores are computed in
                # <=256-col pieces (one psum bank each, ring of 4) so
                # the piece(i+4) <- exp(i) slot-release chain hides
                # behind ~2us of PE run-ahead. ctx consumes the full
                # e tile per step (no split). ctx accumulates with
                # lhsT=[V|1]: psum rows 0:64 = ctx, row 64 = softmax
                # denominator (free). One accumulation group per bank.
                cxp = {0: cx.tile([VS, 2, DG], f32, tag="cx0",
                                  name="cxp0"),
                       1: cx.tile([VS, 2, DG], f32, tag="cx1",
                                  name="cxp1")}
                jmax = {0: 4, 1: 8}
                es = {}

                def emit_scores(n, j):
                    co = max(0, P * j - DG * n)
                    s_ps = sc.tile([P, 2, DG], f32, tag="s",
                                   name=f"s{p}_{n}_{j}")
                    for h in range(2):
                        nc.tensor.matmul(
                            s_ps[:, h, co:DG],
                            kt[:, p, P * j:P * (j + 1)],
                            qz[:, p, h, DG * n + co:DG * (n + 1)],
                            start=True, stop=True,
                            skip_group_check=True)
                    e = ev.tile([P, 2, DG], f16, tag="e",
                                name=f"e{p}_{n}_{j}", bufs=6)
                    nc.scalar.activation(e[:, :, co:DG],
                                         s_ps[:, :, co:DG],
                                         AF.Exp, scale=0.125)
                    # causal mask on the diagonal 128-col block
                    if P * j >= DG * n:
                        nc.vector.tensor_mul(e[:, :, co:co + P],
                                             e[:, :, co:co + P],
                                             tri2[:])
                    es[(n, j)] = e

                def emit_ctx(n, j):
                    co = max(0, P * j - DG * n)
                    e = es.pop((n, j))
                    st, sp = (j == 0), (j == jmax[n] - 1)
                    for h in range(2):
                        hh = 2 * p + h
                        nc.tensor.matmul(
                            cxp[n][:, h, co:DG],
                            v1[:, j, VS * hh:VS * hh + VS],
                            e[:, h, co:DG], start=st, stop=sp,
                            skip_group_check=True)

                def make_norm(n):
                    # normalize, split into small steps sprinkled by the
                    # piece loop (a DVE burst at a pair boundary stalls
                    # the PE and can re-throttle the HAM clock gate).
                    # PE-broadcast the raw f16-staged denominator to 64
                    # rows at base 0, reciprocal there (custom-DVE recip
                    # is base-0 only), multiply; h1's normalized ctx
                    # moves to partitions 64:128 via an identity matmul.
                    # All DVE ops partition-aligned.
                    cxn = cxp[n]
                    qsl = slice(DG * n, DG * (n + 1))
                    st = {}

                    def s0():
                        st["ddf"] = npl.tile([P, 2, DG], f16, tag="ddf",
                                             name="ddf")
                        nc.vector.tensor_copy(st["ddf"][64:65, :, :],
                                              cxn[64:65, :, :])

                    def s1():
                        st["rbd"] = rbd = sc.tile([P, 2, DG], f32,
                                                  tag="s", name="rbd")
                        for h in range(2):
                            nc.tensor.matmul(rbd[0:64, h, :],
                                             ones[64:65, 0:64],
                                             st["ddf"][64:65, h, :],
                                             start=True, stop=True,
                                             tile_position=(64, 0),
                                             skip_group_check=True)

                    def s2():
                        st["rbs"] = rbs = npl.tile([64, 2, DG], f32,
                                                   tag="rbs", name="rbs")
                        nc.vector.reciprocal_approx_fast(
                            rbs[:], st["rbd"][0:64, :, :])

                    def s3():
                        nc.vector.tensor_mul(cn[0:64, p, qsl],
                                             cxn[0:64, 0, :],
                                             st["rbs"][:, 0, :])

                    def s4():
                        st["h1n"] = h1n = npl.tile([64, DG], f16,
                                                   tag="h1n", name="h1n")
                        nc.vector.tensor_mul(h1n[:], cxn[0:64, 1, :],
                                             st["rbs"][:, 1, :])

                    def s5():
                        st["pmv"] = pmv = sc.tile([P, DG], f32, tag="s",
                                                  name="pmv")
                        nc.tensor.matmul(pmv[64:P, :], i64[:],
                                         st["h1n"][:],
                                         start=True, stop=True,
                                         tile_position=(0, 64),
                                         skip_group_check=True)

                    def s6():
                        nc.vector.tensor_copy(cn[64:P, p, qsl],
                                              st["pmv"][64:P, :])

                    return [s0, s1, s2, s3, s4, s5, s6]

                slots = ([(0, j) for j in range(4)]
                         + [(1, j) for j in range(8)])
                LAG = 3
                for i in range(len(slots) + LAG):
                    if i < len(slots):
                        emit_scores(*slots[i])
                    if norm_pend:
                        norm_pend.pop(0)()
                    if i >= LAG:
                        n, j = slots[i - LAG]
                        emit_ctx(n, j)
                        if (n, j) == (0, jmax[0] - 1):
                            norm_pend.extend(make_norm(0))
                norm_pend.extend(make_norm(1))

            # phase A: all projections. K via k-major groups (starts as
            # soon as the first DMAs land), Q with the rope software
            # pipeline, V with ScalarE evac.
            with tc.tile_pool(name="ppA", bufs=8, space="PSUM") as ppA:
                g1 = proj_k_group(ppA, (0, 1))
                for ps, dsts, bias, csl in g1:
                    rope_evac(ps, dsts, bias, csl)
                g2 = proj_k_group(ppA, (2, 3))
                for ps, dsts, bias, csl in g2:
                    rope_evac(ps, dsts, bias, csl)
                for _ in range(4):
                    rope_finish(ppA)
                for t in range(TT):
                    proj_q(ppA, t)
                while rope_pend:
                    rope_finish(ppA)
                for i in range(NT):
                    proj_v(ppA, i)

            # phase B: attention (softmax exp on ScalarE overlaps the
            # PE scores/ctx stream; normalize is half-pipelined)
            with tc.tile_pool(name="sc", bufs=2, space="PSUM") as sc, \
                 tc.tile_pool(name="cx", bufs=1, space="PSUM") as cx:
                for p in range(TT):
                    attn_pair(sc, cx, p)
                while norm_pend:
                    norm_pend.pop(0)()

            if taps:
                for tn, tile_ap in (("qz", qz), ("kt", kt), ("v1", v1),
                                    ("cn", cn)):
                    nc.sync.dma_start(tap_ext[tn][:], tile_ap[:])

            # ---- partial output projection (natural [s, dout]) ----
            with tc.tile_pool(name="op", bufs=4, space="PSUM") as op, \
                 tc.tile_pool(name="ob", bufs=4) as ob:
                # bo is added host-side after summing the two partials
                for i in range(NT):
                    ssl = slice(P * i, P * (i + 1))
                    for c in range(2):
                        csl = slice(DG * c, DG * (c + 1))
                        yp = op.tile([P, DG], f32, tag="yp", name="yp")
                        for t in range(TT):
                            nc.tensor.matmul(yp[:], cn[:, t, ssl],
                                             wo[:, t, csl],
                                             start=(t == 0),
                                             stop=(t == TT - 1))
                        ys = ob.tile([P, DG], f16, tag="ys", name="ys")
                        # alternate evac engines to halve the tail
                        if (2 * i + c) % 2 == 0:
                            nc.scalar.activation(ys[:], yp[:], AF.Copy)
                        else:
                            nc.vector.tensor_copy(ys[:], yp[:])
                        q = (nc.sync, nc.gpsimd)[(2 * i + c) % 2]
                        q.dma_start(y_sh[ssl, csl], ys[:])

    nc.compile()
    return nc


def _host_tables():
    # RoPE tables, computed in float32 to match the reference's jnp path.
    pos = np.arange(S, dtype=np.float32)
    inv = np.exp(np.arange(0, Dh, 2, dtype=np.float32)
                 * np.float32(-np.log(10000.0) / Dh))          # [32]
    ang = pos[:, None] * inv[None, :]                          # [S, 32]
    sin = np.sin(ang).astype(np.float32)
    cos = np.cos(ang).astype(np.float32)
    # per-partition pattern for [2 heads x 64, s] transposed layout
    dd = np.arange(P) % Dh
    cosP = np.empty((P, S), np.float32)
    sinP = np.empty((P, S), np.float32)
    lo = dd < 32
    cosP[lo] = cos[:, dd[lo]].T
    sinP[lo] = -sin[:, dd[lo]].T
    cosP[~lo] = cos[:, dd[~lo] - 32].T
    sinP[~lo] = sin[:, dd[~lo] - 32].T
    return cosP.astype(np.float16), sinP.astype(np.float16)


def _perm128():
    p = np.zeros((P, P), np.float16)
    i = np.arange(P)
    p[i, i ^ 32] = np.float16(1.0)
    return p


def _tile_T(a):
    # [rows, D] -> [NT, P, rows]: k-tile-major transpose (contiguous DMA)
    rows = a.shape[0]
    return np.ascontiguousarray(a.T.reshape(NT, P, rows))


def make_in_maps(x, Wq, bq, Wk, bk, Wv, bv, Wo, bo):
    x = np.asarray(x, np.float16)
    Wq = np.asarray(Wq, np.float16)
    Wk = np.asarray(Wk, np.float16)
    Wv = np.asarray(Wv, np.float16)
    Wo = np.asarray(Wo, np.float16)
    bq = np.asarray(bq, np.float16).astype(np.float32)
    bk = np.asarray(bk, np.float16).astype(np.float32)
    cosP, sinP = _host_tables()
    r = np.arange(P)[:, None]
    c = np.arange(P)[None, :]
    tri = (c >= r).astype(np.float16)                     # [kv, q] valid
    tri2 = np.ascontiguousarray(
        np.broadcast_to(tri[:, None, :], (P, 2, P)))
    shared = {
        "cosk": cosP, "sink": sinP, "tri2": tri2, "p128": _perm128(),
        "i64": np.eye(64, dtype=np.float16),
    }
    xt_by_batch = [_tile_T(x[b]) for b in range(B)]

    in_maps = []
    for core in range(N_CORES):
        b, g = core // 2, core % 2
        gsl = slice(DG * g, DG * (g + 1))
        m = {
            "xt_sh": xt_by_batch[b],
            "wq": np.ascontiguousarray(Wq[:, gsl]),
            "wk": np.ascontiguousarray(Wk[:, gsl]),
            "wv": np.ascontiguousarray(Wv[:, gsl]),
            "wo": np.ascontiguousarray(Wo[gsl, :]),
            "bqt": np.ascontiguousarray(bq[gsl].reshape(TT, P).T),
            "bkt": np.ascontiguousarray(bk[gsl].reshape(TT, P).T),
            "bv": np.asarray(bv, np.float16)[gsl].reshape(1, DG),
        }
        m.update(shared)
        in_maps.append(m)
    return in_maps


def kernel(x, Wq, bq, Wk, bk, Wv, bv, Wo, bo):
    from concourse.bass_utils import run_bass_kernel_spmd

    with _lock:
        if "nc" not in _cache:
            _cache["nc"] = _build_program()
    nc = _cache["nc"]

    in_maps = make_in_maps(x, Wq, bq, Wk, bk, Wv, bv, Wo, bo)
    res = run_bass_kernel_spmd(nc, in_maps, list(range(N_CORES)))

    out = np.empty((B, S, D), np.float16)
    bo_f = np.asarray(bo, np.float32).reshape(1, D)
    for b in range(B):
        y0 = res.results[2 * b]["y_sh"].astype(np.float32)
        y1 = res.results[2 * b + 1]["y_sh"].astype(np.float32)
        out[b] = (y0 + y1 + bo_f).astype(np.float16)
    return out


# revision 83
# speedup vs baseline: 1.0926x; 1.0926x over previous
"""Trainium2 Bass kernel for CustomMultiHeadAttention (B=4, S=1024, D=1024, H=16, Dh=64).

Sharding: 8 cores = (batch b in 0..3) x (head-group g in 0..1).
Core (b, g) computes heads 8g..8g+7 for ALL 1024 positions of batch b:
  - Q/K/V projections use only the 512 dout columns of Wq/Wk/Wv for its heads
  - attention (causal softmax) for its 8 heads over the full sequence
  - a PARTIAL output projection y_part = ctx_g @ Wo[512g:512(g+1), :]
The host sums the two partial outputs per batch (free for HW time).

Vs batch x parity sharding this halves every projection's per-core work
(no K/V duplication), halves weight DMA (4MB vs 8MB), and keeps the
causal mask a single constant lower-tri block.

Pipeline (transposed layout, PE-centric):
  KT = rope(Wk^T x^T), QT = rope(Wq^T x^T)  - rope via perm-matmul + DVE
  V in natural [s, dout] 65-wide slots [V(64) | ones(1)] per head
  scores sc[kv, q] = KT_h^T QT_h per 128-kv block j, q processed in two
  512-col halves; exp on ScalarE (scale=1/8); causal mask = tri multiply
  on the diagonal block; ctx accumulates with lhsT=[V|1] so psum row 64
  is the softmax denominator (free); normalize via reciprocal + PE
  broadcast; y_part = ctx^T Wo_half (natural layout, DMA out).
"""

import threading

import numpy as np

B, S, D, H, Dh = 4, 1024, 1024, 16, 64
P = 128
N_CORES = 8
NT = D // P    # 8 k-tiles along din
TT = 4         # dout-half tiles (512 / 128)
DG = 512       # dout per head group
VS = 65        # V slot width: [V(64) | ones(1)] per head

_cache = {}
_lock = threading.Lock()


def _build_program(taps=False):
    import concourse.bass as bass  # noqa: F401
    import concourse.mybir as mybir
    import concourse.tile as tile
    from concourse import bacc

    dt = mybir.dt
    f16, f32 = dt.float16, dt.float32
    AF = mybir.ActivationFunctionType

    nc = bacc.Bacc("TRN2", target_bir_lowering=False, debug=False,
                   num_devices=N_CORES)

    def ein(name, shape):
        return nc.dram_tensor(name, shape, f16, kind="ExternalInput").ap()

    xt_sh = ein("xt_sh", [NT, P, S])      # x[b]^T, host-transposed
                                          # (k-tile major: contiguous DMA)
    wq_e = ein("wq", [D, DG])             # Wq[:, 512g:512(g+1)]
    wk_e = ein("wk", [D, DG])
    wv_e = ein("wv", [D, DG])
    wo_e = ein("wo", [DG, D])             # Wo[512g:512(g+1), :]
    bqt_e = nc.dram_tensor("bqt", [P, TT], f32, kind="ExternalInput").ap()
    bkt_e = nc.dram_tensor("bkt", [P, TT], f32, kind="ExternalInput").ap()
    bv_e = ein("bv", [1, DG])
    cosk_e = ein("cosk", [P, S])
    sink_e = ein("sink", [P, S])
    tri2_e = ein("tri2", [P, 2, P])       # causal mask, replicated x2
    p128_e = ein("p128", [P, P])
    i64_e = ein("i64", [64, 64])
    y_sh = nc.dram_tensor("y_sh", [S, D], f16, kind="ExternalOutput").ap()
    tap_ext = {}
    if taps:
        for tn, shape in (("qz", [P, TT, 2, S]), ("kt", [P, TT, S]),
                          ("v1", [P, NT, 8 * VS]), ("cn", [P, TT, S])):
            tap_ext[tn] = nc.dram_tensor("dbg_" + tn, shape, f16,
                                         kind="ExternalOutput").ap()

    with tile.TileContext(nc) as tc:
        from contextlib import ExitStack
        with ExitStack() as ctx:
            big = ctx.enter_context(tc.tile_pool(name="big", bufs=1))

            xT = big.tile([P, NT, S], f16, tag="xT")       # x[b]^T [din, s]
            wq = big.tile([P, NT, DG], f16, tag="wq")
            wk = big.tile([P, NT, DG], f16, tag="wk")
            wv = big.tile([P, NT, DG], f16, tag="wv")
            wo = big.tile([P, TT, D], f16, tag="wo")
            bqt = big.tile([P, TT], f32, tag="bqt")
            bkt = big.tile([P, TT], f32, tag="bkt")
            bv_sb = big.tile([1, DG], f16, tag="bv")
            # rope'd Q^T, per-head zero-padded: qz[0:64, p, 0] = head 2p,
            # qz[64:128, p, 1] = head 2p+1, other halves zero. Scores use
            # the full-128-row kt tile as a SHARED lhsT for both heads;
            # the zero half of qz kills the other head's contribution.
            # (Keeps every attention matmul in plain 128-row mode: the
            # 64-row T8-tiled scores + 65-wide ctx combination is fatal
            # on HW.)
            qz = big.tile([P, TT, 2, S], f16, tag="qz")
            kt = big.tile([P, TT, S], f16, tag="kt")       # rope'd K^T
            v1 = big.tile([P, NT, 8 * VS], f16, tag="v1")  # [V|1] slots
            cn = big.tile([P, TT, S], f16, tag="cn")       # normalized ctx^T
            cosk = big.tile([P, S], f16, tag="cosk")
            sink = big.tile([P, S], f16, tag="sink")
            tri2 = big.tile([P, 2, P], f16, tag="tri2")
            p128 = big.tile([P, P], f16, tag="p128")
            i64 = big.tile([64, 64], f16, tag="i64")
            ones = big.tile([P, DG], f16, tag="ones")
            warm = big.tile([1, 16], f16, tag="warm")

            # ---- input DMAs ----
            # the K-proj critical path needs (xT[k], wk[k]) pairs as
            # early as possible; give xT the sync queue to itself and
            # wk the scalar queue so the two streams run in parallel.
            for k in range(NT):
                nc.sync.dma_start(xT[:, k, :], xt_sh[k])
                nc.scalar.dma_start(wk[:, k, :],
                                    wk_e[P * k:P * (k + 1), :])
                if k == 0:
                    for t, e in ((p128, p128_e), (bkt, bkt_e)):
                        nc.gpsimd.dma_start(t[:], e[:])
                if k == 1:
                    for t, e in ((cosk, cosk_e), (sink, sink_e)):
                        nc.gpsimd.dma_start(t[:], e[:])
            for k in range(NT):
                nc.gpsimd.dma_start(wv[:, k, :],
                                    wv_e[P * k:P * (k + 1), :])
            nc.gpsimd.dma_start(bv_sb[:], bv_e[:])
            for k in range(NT):
                nc.sync.dma_start(wq[:, k, :], wq_e[P * k:P * (k + 1), :])
                if k == 0:
                    nc.sync.dma_start(bqt[:], bqt_e[:])
            nc.scalar.dma_start(tri2[:], tri2_e[:])
            nc.scalar.dma_start(i64[:], i64_e[:])
            for t in range(TT):
                nc.scalar.dma_start(wo[:, t, :],
                                    wo_e[P * t:P * (t + 1), :])

            nc.vector.memset(qz[:], 0.0)
            nc.any.memset(ones[:], 1.0)
            v1r = v1.rearrange("p t (h c) -> p t h c", c=VS)
            for t in range(NT):
                nc.any.memset(v1r[:, t, :, 64:65], 1.0)
            # preload the exp table on ScalarE so the first real exp
            # doesn't pay ACT_TABLE_LOAD on the critical path
            nc.scalar.activation(warm[:], ones[0:1, 0:16], AF.Exp, scale=0.01)

            # ---- projections + rope + attention, phased pools ----
            ev = ctx.enter_context(tc.tile_pool(name="ev", bufs=3))
            npl = ctx.enter_context(tc.tile_pool(name="npl", bufs=2))

            # rope is emitted in two stages with a 1-chunk software
            # pipeline: the perm matmul of chunk c is issued after chunk
            # c+1's k-chain so the in-order PE queue never waits on the
            # DVE evac of chunk c.
            rope_pend = []

            def rope_finish(pp):
                if not rope_pend:
                    return
                raw, dsts, csl = rope_pend.pop(0)
                pq = pp.tile([P, DG], f32, tag="ps", name="pq")
                nc.tensor.matmul(pq[:], p128[:], raw[:],
                                 start=True, stop=True)
                t1 = ev.tile([P, DG], f16, tag="t1", name="t1")
                nc.vector.tensor_mul(t1[:], raw[:], cosk[:, csl])
                t2 = ev.tile([P, DG], f16, tag="t2", name="t2")
                nc.vector.tensor_mul(t2[:], pq[:], sink[:, csl])
                for rs, dst in dsts:
                    nc.vector.tensor_add(dst, t1[rs, :], t2[rs, :])

            norm_pend = []

            def rope_evac(ps, dsts, bias, csl):
                # psum evac with fused per-partition bias add
                raw = ev.tile([P, DG], f16, tag="raw", name="raw",
                              bufs=12)
                nc.vector.tensor_scalar_add(raw[:], ps[:], bias)
                rope_pend.append((raw, dsts, csl))

            def proj_k_group(pp, ts):
                # k-major accumulation over 2 dout tiles x 2 s-chunks so
                # the chains start as soon as the first (xT, wk) DMA
                # pair lands instead of waiting for all of wk
                chunks = [(t, n2) for t in ts for n2 in range(2)]
                cps = {c: pp.tile([P, DG], f32, tag="ps",
                                  name=f"kp{c[0]}{c[1]}") for c in chunks}
                for k in range(NT):
                    for (t, n2) in chunks:
                        nc.tensor.matmul(cps[(t, n2)][:],
                                         wk[:, k, P * t:P * (t + 1)],
                                         xT[:, k,
                                            DG * n2:DG * (n2 + 1)],
                                         start=(k == 0),
                                         stop=(k == NT - 1))
                return [(cps[(t, n2)],
                         [(slice(0, P), kt[:, t,
                                          DG * n2:DG * (n2 + 1)])],
                         bkt[:, t:t + 1],
                         slice(DG * n2, DG * (n2 + 1)))
                        for (t, n2) in chunks]

            def proj_q(pp, t):
                wsl = slice(P * t, P * (t + 1))
                for n in range(2):
                    csl = slice(DG * n, DG * (n + 1))
                    ps = pp.tile([P, DG], f32, tag="ps", name="qp")
                    for k in range(NT):
                        nc.tensor.matmul(ps[:], wq[:, k, wsl],
                                         xT[:, k, csl],
                                         start=(k == 0),
                                         stop=(k == NT - 1))
                    rope_evac(ps,
                              [(slice(0, 64), qz[0:64, t, 0, csl]),
                               (slice(64, P), qz[64:P, t, 1, csl])],
                              bqt[:, t:t + 1], csl)
                    rope_finish(pp)

            def proj_v(pp, i):
                # V s-block i: natural [s, dout] into 65-wide slots;
                # evac on ScalarE (idle during proj) to keep DVE free
                ssl = slice(P * i, P * (i + 1))
                vp = pp.tile([P, DG], f32, tag="ps", name="vp")
                for k in range(NT):
                    nc.tensor.matmul(vp[:], xT[:, k, ssl], wv[:, k, :],
                                     start=(k == 0), stop=False)
                nc.tensor.matmul(vp[:], ones[0:1, 0:P], bv_sb[0:1, :],
                                 start=False, stop=True)
                nc.scalar.activation(
                    v1r[:, i, :, 0:64],
                    vp.rearrange("p (h c) -> p h c", c=64), AF.Copy)

            def attn_pair(sc, cx, p):
                # heads h0 = 2p, h1 = 2p+1. Scores are computed in
                # <=256-col pieces (one psum bank each, ring of 4) so
                # the piece(i+4) <- exp(i) slot-release chain hides
                # behind ~2us of PE run-ahead. ctx consumes the full
                # e tile per step (no split). ctx accumulates with
                # lhsT=[V|1]: psum rows 0:64 = ctx, row 64 = softmax
                # denominator (free). One accumulation group per bank.
                cxp = {0: cx.tile([VS, 2, DG], f32, tag="cx0",
                                  name="cxp0"),
                       1: cx.tile([VS, 2, DG], f32, tag="cx1",
                                  name="cxp1")}
                jmax = {0: 4, 1: 8}
                es = {}

                def emit_scores(n, j):
                    co = max(0, P * j - DG * n)
                    s_ps = sc.tile([P, 2, DG], f32, tag="s",
                                   name=f"s{p}_{n}_{j}")
                    for h in range(2):
                        nc.tensor.matmul(
                            s_ps[:, h, co:DG],
                            kt[:, p, P * j:P * (j + 1)],
                            qz[:, p, h, DG * n + co:DG * (n + 1)],
                            start=True, stop=True,
                            skip_group_check=True)
                    e = ev.tile([P, 2, DG], f16, tag="e",
                                name=f"e{p}_{n}_{j}", bufs=6)
                    nc.scalar.activation(e[:, :, co:DG],
                                         s_ps[:, :, co:DG],
                                         AF.Exp, scale=0.125)
                    # causal mask on the diagonal 128-col block
                    if P * j >= DG * n:
                        nc.vector.tensor_mul(e[:, :, co:co + P],
                                             e[:, :, co:co + P],
                                             tri2[:])
                    es[(n, j)] = e

                def emit_ctx(n, j):
                    co = max(0, P * j - DG * n)
                    e = es.pop((n, j))
                    st, sp = (j == 0), (j == jmax[n] - 1)
                    for h in range(2):
                        hh = 2 * p + h
                        nc.tensor.matmul(
                            cxp[n][:, h, co:DG],
                            v1[:, j, VS * hh:VS * hh + VS],
                            e[:, h, co:DG], start=st, stop=sp,
                            skip_group_check=True)

                def make_norm(n):
                    # normalize, split into small steps sprinkled by the
                    # piece loop (a DVE burst at a pair boundary stalls
                    # the PE and can re-throttle the HAM clock gate).
                    # PE-broadcast the raw f16-staged denominator to 64
                    # rows at base 0, reciprocal there (custom-DVE recip
                    # is base-0 only), multiply; h1's normalized ctx
                    # moves to partitions 64:128 via an identity matmul.
                    # All DVE ops partition-aligned.
                    cxn = cxp[n]
                    qsl = slice(DG * n, DG * (n + 1))
                    st = {}

                    def s0():
                        st["ddf"] = npl.tile([P, 2, DG], f16, tag="ddf",
                                             name="ddf")
                        nc.vector.tensor_copy(st["ddf"][64:65, :, :],
                                              cxn[64:65, :, :])

                    def s1():
                        st["rbd"] = rbd = sc.tile([P, 2, DG], f32,
                                                  tag="s", name="rbd")
                        for h in range(2):
                            nc.tensor.matmul(rbd[0:64, h, :],
                                             ones[64:65, 0:64],
                                             st["ddf"][64:65, h, :],
                                             start=True, stop=True,
                                             tile_position=(64, 0),
                                             skip_group_check=True)

                    def s2():
                        st["rbs"] = rbs = npl.tile([64, 2, DG], f32,
                                                   tag="rbs", name="rbs")
                        nc.vector.reciprocal_approx_fast(
                            rbs[:], st["rbd"][0:64, :, :])

                    def s3():
                        nc.vector.tensor_mul(cn[0:64, p, qsl],
                                             cxn[0:64, 0, :],
                                             st["rbs"][:, 0, :])

                    def s4():
                        st["h1n"] = h1n = npl.tile([64, DG], f16,
                                                   tag="h1n", name="h1n")
                        nc.vector.tensor_mul(h1n[:], cxn[0:64, 1, :],
                                             st["rbs"][:, 1, :])

                    def s5():
                        st["pmv"] = pmv = sc.tile([P, DG], f32, tag="s",
                                                  name="pmv")
                        nc.tensor.matmul(pmv[64:P, :], i64[:],
                                         st["h1n"][:],
                                         start=True, stop=True,
                                         tile_position=(0, 64),
                                         skip_group_check=True)

                    def s6():
                        nc.vector.tensor_copy(cn[64:P, p, qsl],
                                              st["pmv"][64:P, :])

                    return [s0, s1, s2, s3, s4, s5, s6]

                slots = ([(0, j) for j in range(4)]
                         + [(1, j) for j in range(8)])
                LAG = 3
                for i in range(len(slots) + LAG):
                    if i < len(slots):
                        emit_scores(*slots[i])
                    if norm_pend:
                        norm_pend.pop(0)()
                    if i >= LAG:
                        n, j = slots[i - LAG]
                        emit_ctx(n, j)
                        if (n, j) == (0, jmax[0] - 1):
                            norm_pend.extend(make_norm(0))
                norm_pend.extend(make_norm(1))

            # phase A: all projections. K via k-major groups (starts as
            # soon as the first DMAs land), Q with the rope software
            # pipeline, V with ScalarE evac.
            with tc.tile_pool(name="ppA", bufs=8, space="PSUM") as ppA:
                g1 = proj_k_group(ppA, (0, 1, 2, 3))
                for ps, dsts, bias, csl in g1:
                    rope_evac(ps, dsts, bias, csl)
                for t in range(TT):
                    proj_q(ppA, t)
                while rope_pend:
                    rope_finish(ppA)
                for i in range(NT):
                    proj_v(ppA, i)

            # phase B: attention (softmax exp on ScalarE overlaps the
            # PE scores/ctx stream; normalize is half-pipelined)
            with tc.tile_pool(name="sc", bufs=2, space="PSUM") as sc, \
                 tc.tile_pool(name="cx", bufs=1, space="PSUM") as cx:
                for p in range(TT):
                    attn_pair(sc, cx, p)
                while norm_pend:
                    norm_pend.pop(0)()

            if taps:
                for tn, tile_ap in (("qz", qz), ("kt", kt), ("v1", v1),
                                    ("cn", cn)):
                    nc.sync.dma_start(tap_ext[tn][:], tile_ap[:])

            # ---- partial output projection (natural [s, dout]) ----
            with tc.tile_pool(name="op", bufs=4, space="PSUM") as op, \
                 tc.tile_pool(name="ob", bufs=4) as ob:
                # bo is added host-side after summing the two partials
                for i in range(NT):
                    ssl = slice(P * i, P * (i + 1))
                    for c in range(2):
                        csl = slice(DG * c, DG * (c + 1))
                        yp = op.tile([P, DG], f32, tag="yp", name="yp")
                        for t in range(TT):
                            nc.tensor.matmul(yp[:], cn[:, t, ssl],
                                             wo[:, t, csl],
                                             start=(t == 0),
                                             stop=(t == TT - 1))
                        ys = ob.tile([P, DG], f16, tag="ys", name="ys")
                        # alternate evac engines to halve the tail
                        if (2 * i + c) % 2 == 0:
                            nc.scalar.activation(ys[:], yp[:], AF.Copy)
                        else:
                            nc.vector.tensor_copy(ys[:], yp[:])
                        nc.sync.dma_start(y_sh[ssl, csl], ys[:])

    nc.compile()
    return nc


def _host_tables():
    # RoPE tables, computed in float32 to match the reference's jnp path.
    pos = np.arange(S, dtype=np.float32)
    inv = np.exp(np.arange(0, Dh, 2, dtype=np.float32)
                 * np.float32(-np.log(10000.0) / Dh))          # [32]
    ang = pos[:, None] * inv[None, :]                          # [S, 32]
    sin = np.sin(ang).astype(np.float32)
    cos = np.cos(ang).astype(np.float32)
    # per-partition pattern for [2 heads x 64, s] transposed layout
    dd = np.arange(P) % Dh
    cosP = np.empty((P, S), np.float32)
    sinP = np.empty((P, S), np.float32)
    lo = dd < 32
    cosP[lo] = cos[:, dd[lo]].T
    sinP[lo] = -sin[:, dd[lo]].T
    cosP[~lo] = cos[:, dd[~lo] - 32].T
    sinP[~lo] = sin[:, dd[~lo] - 32].T
    return cosP.astype(np.float16), sinP.astype(np.float16)


def _perm128():
    p = np.zeros((P, P), np.float16)
    i = np.arange(P)
    p[i, i ^ 32] = np.float16(1.0)
    return p


def _tile_T(a):
    # [rows, D] -> [NT, P, rows]: k-tile-major transpose (contiguous DMA)
    rows = a.shape[0]
    return np.ascontiguousarray(a.T.reshape(NT, P, rows))


def make_in_maps(x, Wq, bq, Wk, bk, Wv, bv, Wo, bo):
    x = np.asarray(x, np.float16)
    Wq = np.asarray(Wq, np.float16)
    Wk = np.asarray(Wk, np.float16)
    Wv = np.asarray(Wv, np.float16)
    Wo = np.asarray(Wo, np.float16)
    bq = np.asarray(bq, np.float16).astype(np.float32)
    bk = np.asarray(bk, np.float16).astype(np.float32)
    cosP, sinP = _host_tables()
    r = np.arange(P)[:, None]
    c = np.arange(P)[None, :]
    tri = (c >= r).astype(np.float16)                     # [kv, q] valid
    tri2 = np.ascontiguousarray(
        np.broadcast_to(tri[:, None, :], (P, 2, P)))
    shared = {
        "cosk": cosP, "sink": sinP, "tri2": tri2, "p128": _perm128(),
        "i64": np.eye(64, dtype=np.float16),
    }
    xt_by_batch = [_tile_T(x[b]) for b in range(B)]

    in_maps = []
    for core in range(N_CORES):
        b, g = core // 2, core % 2
        gsl = slice(DG * g, DG * (g + 1))
        m = {
            "xt_sh": xt_by_batch[b],
            "wq": np.ascontiguousarray(Wq[:, gsl]),
            "wk": np.ascontiguousarray(Wk[:, gsl]),
            "wv": np.ascontiguousarray(Wv[:, gsl]),
            "wo": np.ascontiguousarray(Wo[gsl, :]),
            "bqt": np.ascontiguousarray(bq[gsl].reshape(TT, P).T),
            "bkt": np.ascontiguousarray(bk[gsl].reshape(TT, P).T),
            "bv": np.asarray(bv, np.float16)[gsl].reshape(1, DG),
        }
        m.update(shared)
        in_maps.append(m)
    return in_maps


def kernel(x, Wq, bq, Wk, bk, Wv, bv, Wo, bo):
    from concourse.bass_utils import run_bass_kernel_spmd

    with _lock:
        if "nc" not in _cache:
            _cache["nc"] = _build_program()
    nc = _cache["nc"]

    in_maps = make_in_maps(x, Wq, bq, Wk, bk, Wv, bv, Wo, bo)
    res = run_bass_kernel_spmd(nc, in_maps, list(range(N_CORES)))

    out = np.empty((B, S, D), np.float16)
    bo_f = np.asarray(bo, np.float32).reshape(1, D)
    for b in range(B):
        y0 = res.results[2 * b]["y_sh"].astype(np.float32)
        y1 = res.results[2 * b + 1]["y_sh"].astype(np.float32)
        out[b] = (y0 + y1 + bo_f).astype(np.float16)
    return out


# revision 84
# speedup vs baseline: 1.1051x; 1.0115x over previous
"""Trainium2 Bass kernel for CustomMultiHeadAttention (B=4, S=1024, D=1024, H=16, Dh=64).

Sharding: 8 cores = (batch b in 0..3) x (head-group g in 0..1).
Core (b, g) computes heads 8g..8g+7 for ALL 1024 positions of batch b:
  - Q/K/V projections use only the 512 dout columns of Wq/Wk/Wv for its heads
  - attention (causal softmax) for its 8 heads over the full sequence
  - a PARTIAL output projection y_part = ctx_g @ Wo[512g:512(g+1), :]
The host sums the two partial outputs per batch (free for HW time).

Vs batch x parity sharding this halves every projection's per-core work
(no K/V duplication), halves weight DMA (4MB vs 8MB), and keeps the
causal mask a single constant lower-tri block.

Pipeline (transposed layout, PE-centric):
  KT = rope(Wk^T x^T), QT = rope(Wq^T x^T)  - rope via perm-matmul + DVE
  V in natural [s, dout] 65-wide slots [V(64) | ones(1)] per head
  scores sc[kv, q] = KT_h^T QT_h per 128-kv block j, q processed in two
  512-col halves; exp on ScalarE (scale=1/8); causal mask = tri multiply
  on the diagonal block; ctx accumulates with lhsT=[V|1] so psum row 64
  is the softmax denominator (free); normalize via reciprocal + PE
  broadcast; y_part = ctx^T Wo_half (natural layout, DMA out).
"""

import threading

import numpy as np

B, S, D, H, Dh = 4, 1024, 1024, 16, 64
P = 128
N_CORES = 8
NT = D // P    # 8 k-tiles along din
TT = 4         # dout-half tiles (512 / 128)
DG = 512       # dout per head group
VS = 65        # V slot width: [V(64) | ones(1)] per head

_cache = {}
_lock = threading.Lock()


def _build_program(taps=False):
    import concourse.bass as bass  # noqa: F401
    import concourse.mybir as mybir
    import concourse.tile as tile
    from concourse import bacc

    dt = mybir.dt
    f16, f32 = dt.float16, dt.float32
    AF = mybir.ActivationFunctionType

    nc = bacc.Bacc("TRN2", target_bir_lowering=False, debug=False,
                   num_devices=N_CORES)

    def ein(name, shape):
        return nc.dram_tensor(name, shape, f16, kind="ExternalInput").ap()

    xt_sh = ein("xt_sh", [NT, P, S])      # x[b]^T, host-transposed
                                          # (k-tile major: contiguous DMA)
    wq_e = ein("wq", [D, DG])             # Wq[:, 512g:512(g+1)]
    wk_e = ein("wk", [D, DG])
    wv_e = ein("wv", [D, DG])
    wo_e = ein("wo", [DG, D])             # Wo[512g:512(g+1), :]
    bqt_e = nc.dram_tensor("bqt", [P, TT], f32, kind="ExternalInput").ap()
    bkt_e = nc.dram_tensor("bkt", [P, TT], f32, kind="ExternalInput").ap()
    bv_e = ein("bv", [1, DG])
    cosk_e = ein("cosk", [P, S])
    sink_e = ein("sink", [P, S])
    tri2_e = ein("tri2", [P, 2, P])       # causal mask, replicated x2
    p128_e = ein("p128", [P, P])
    i64_e = ein("i64", [64, 64])
    y_sh = nc.dram_tensor("y_sh", [S, D], f16, kind="ExternalOutput").ap()
    tap_ext = {}
    if taps:
        for tn, shape in (("qz", [P, TT, 2, S]), ("kt", [P, TT, S]),
                          ("v1", [P, NT, 8 * VS]), ("cn", [P, TT, S])):
            tap_ext[tn] = nc.dram_tensor("dbg_" + tn, shape, f16,
                                         kind="ExternalOutput").ap()

    with tile.TileContext(nc) as tc:
        from contextlib import ExitStack
        with ExitStack() as ctx:
            big = ctx.enter_context(tc.tile_pool(name="big", bufs=1))

            xT = big.tile([P, NT, S], f16, tag="xT")       # x[b]^T [din, s]
            wq = big.tile([P, NT, DG], f16, tag="wq")
            wk = big.tile([P, NT, DG], f16, tag="wk")
            wv = big.tile([P, NT, DG], f16, tag="wv")
            wo = big.tile([P, TT, D], f16, tag="wo")
            bqt = big.tile([P, TT], f32, tag="bqt")
            bkt = big.tile([P, TT], f32, tag="bkt")
            bv_sb = big.tile([1, DG], f16, tag="bv")
            # rope'd Q^T, per-head zero-padded: qz[0:64, p, 0] = head 2p,
            # qz[64:128, p, 1] = head 2p+1, other halves zero. Scores use
            # the full-128-row kt tile as a SHARED lhsT for both heads;
            # the zero half of qz kills the other head's contribution.
            # (Keeps every attention matmul in plain 128-row mode: the
            # 64-row T8-tiled scores + 65-wide ctx combination is fatal
            # on HW.)
            qz = big.tile([P, TT, 2, S], f16, tag="qz")
            kt = big.tile([P, TT, S], f16, tag="kt")       # rope'd K^T
            v1 = big.tile([P, NT, 8 * VS], f16, tag="v1")  # [V|1] slots
            cn = big.tile([P, TT, S], f16, tag="cn")       # normalized ctx^T
            cosk = big.tile([P, S], f16, tag="cosk")
            sink = big.tile([P, S], f16, tag="sink")
            tri2 = big.tile([P, 2, P], f16, tag="tri2")
            p128 = big.tile([P, P], f16, tag="p128")
            i64 = big.tile([64, 64], f16, tag="i64")
            ones = big.tile([P, DG], f16, tag="ones")
            warm = big.tile([1, 16], f16, tag="warm")

            # ---- input DMAs ----
            # the K-proj critical path needs (xT[k], wk[k]) pairs as
            # early as possible; give xT the sync queue to itself and
            # wk the scalar queue so the two streams run in parallel.
            for k in range(NT):
                nc.sync.dma_start(xT[:, k, :], xt_sh[k])
                nc.scalar.dma_start(wk[:, k, :],
                                    wk_e[P * k:P * (k + 1), :])
                if k == 0:
                    for t, e in ((p128, p128_e), (bkt, bkt_e)):
                        nc.gpsimd.dma_start(t[:], e[:])
                if k == 1:
                    for t, e in ((cosk, cosk_e), (sink, sink_e)):
                        nc.gpsimd.dma_start(t[:], e[:])
            for k in range(NT):
                nc.gpsimd.dma_start(wv[:, k, :],
                                    wv_e[P * k:P * (k + 1), :])
            nc.gpsimd.dma_start(bv_sb[:], bv_e[:])
            for k in range(NT):
                nc.sync.dma_start(wq[:, k, :], wq_e[P * k:P * (k + 1), :])
                if k == 0:
                    nc.sync.dma_start(bqt[:], bqt_e[:])
            nc.scalar.dma_start(tri2[:], tri2_e[:])
            nc.scalar.dma_start(i64[:], i64_e[:])
            for t in range(TT):
                nc.scalar.dma_start(wo[:, t, :],
                                    wo_e[P * t:P * (t + 1), :])

            nc.vector.memset(qz[:], 0.0)
            nc.any.memset(ones[:], 1.0)
            v1r = v1.rearrange("p t (h c) -> p t h c", c=VS)
            for t in range(NT):
                nc.any.memset(v1r[:, t, :, 64:65], 1.0)
            # preload the exp table on ScalarE so the first real exp
            # doesn't pay ACT_TABLE_LOAD on the critical path
            nc.scalar.activation(warm[:], ones[0:1, 0:16], AF.Exp, scale=0.01)

            # ---- projections + rope + attention, phased pools ----
            ev = ctx.enter_context(tc.tile_pool(name="ev", bufs=3))
            npl = ctx.enter_context(tc.tile_pool(name="npl", bufs=2))

            # rope is emitted in two stages with a 1-chunk software
            # pipeline: the perm matmul of chunk c is issued after chunk
            # c+1's k-chain so the in-order PE queue never waits on the
            # DVE evac of chunk c.
            rope_pend = []

            def rope_finish(pp):
                if not rope_pend:
                    return
                raw, dsts, csl = rope_pend.pop(0)
                pq = pp.tile([P, DG], f32, tag="ps", name="pq")
                nc.tensor.matmul(pq[:], p128[:], raw[:],
                                 start=True, stop=True)
                t1 = ev.tile([P, DG], f16, tag="t1", name="t1")
                nc.vector.tensor_mul(t1[:], raw[:], cosk[:, csl])
                t2 = ev.tile([P, DG], f16, tag="t2", name="t2")
                nc.vector.tensor_mul(t2[:], pq[:], sink[:, csl])
                for rs, dst in dsts:
                    nc.vector.tensor_add(dst, t1[rs, :], t2[rs, :])

            norm_pend = []

            def rope_evac(ps, dsts, bias, csl):
                # psum evac with fused per-partition bias add
                raw = ev.tile([P, DG], f16, tag="raw", name="raw",
                              bufs=12)
                nc.vector.tensor_scalar_add(raw[:], ps[:], bias)
                rope_pend.append((raw, dsts, csl))

            def proj_k_group(pp, ts):
                # k-major accumulation over 2 dout tiles x 2 s-chunks so
                # the chains start as soon as the first (xT, wk) DMA
                # pair lands instead of waiting for all of wk
                chunks = [(t, n2) for t in ts for n2 in range(2)]
                cps = {c: pp.tile([P, DG], f32, tag="ps",
                                  name=f"kp{c[0]}{c[1]}") for c in chunks}
                for k in range(NT):
                    for (t, n2) in chunks:
                        nc.tensor.matmul(cps[(t, n2)][:],
                                         wk[:, k, P * t:P * (t + 1)],
                                         xT[:, k,
                                            DG * n2:DG * (n2 + 1)],
                                         start=(k == 0),
                                         stop=(k == NT - 1))
                return [(cps[(t, n2)],
                         [(slice(0, P), kt[:, t,
                                          DG * n2:DG * (n2 + 1)])],
                         bkt[:, t:t + 1],
                         slice(DG * n2, DG * (n2 + 1)))
                        for (t, n2) in chunks]

            def proj_q(pp, t):
                wsl = slice(P * t, P * (t + 1))
                for n in range(2):
                    csl = slice(DG * n, DG * (n + 1))
                    ps = pp.tile([P, DG], f32, tag="ps", name="qp")
                    for k in range(NT):
                        nc.tensor.matmul(ps[:], wq[:, k, wsl],
                                         xT[:, k, csl],
                                         start=(k == 0),
                                         stop=(k == NT - 1))
                    rope_evac(ps,
                              [(slice(0, 64), qz[0:64, t, 0, csl]),
                               (slice(64, P), qz[64:P, t, 1, csl])],
                              bqt[:, t:t + 1], csl)
                    rope_finish(pp)

            def proj_v(pp, i):
                # V s-block i: natural [s, dout] into 65-wide slots;
                # evac on ScalarE (idle during proj) to keep DVE free
                ssl = slice(P * i, P * (i + 1))
                vp = pp.tile([P, DG], f32, tag="ps", name="vp")
                for k in range(NT):
                    nc.tensor.matmul(vp[:], xT[:, k, ssl], wv[:, k, :],
                                     start=(k == 0), stop=False)
                nc.tensor.matmul(vp[:], ones[0:1, 0:P], bv_sb[0:1, :],
                                 start=False, stop=True)
                nc.scalar.activation(
                    v1r[:, i, :, 0:64],
                    vp.rearrange("p (h c) -> p h c", c=64), AF.Copy)

            def attn_pair(sc, cx, p):
                # heads h0 = 2p, h1 = 2p+1. Scores are computed in
                # <=256-col pieces (one psum bank each, ring of 4) so
                # the piece(i+4) <- exp(i) slot-release chain hides
                # behind ~2us of PE run-ahead. ctx consumes the full
                # e tile per step (no split). ctx accumulates with
                # lhsT=[V|1]: psum rows 0:64 = ctx, row 64 = softmax
                # denominator (free). One accumulation group per bank.
                cxp = {0: cx.tile([VS, 2, DG], f32, tag="cx0",
                                  name="cxp0"),
                       1: cx.tile([VS, 2, DG], f32, tag="cx1",
                                  name="cxp1")}
                jmax = {0: 4, 1: 8}
                es = {}

                def emit_scores(n, j):
                    co = max(0, P * j - DG * n)
                    s_ps = sc.tile([P, 2, DG], f32, tag="s",
                                   name=f"s{p}_{n}_{j}")
                    for h in range(2):
                        nc.tensor.matmul(
                            s_ps[:, h, co:DG],
                            kt[:, p, P * j:P * (j + 1)],
                            qz[:, p, h, DG * n + co:DG * (n + 1)],
                            start=True, stop=True,
                            skip_group_check=True)
                    e = ev.tile([P, 2, DG], f16, tag="e",
                                name=f"e{p}_{n}_{j}", bufs=6)
                    nc.scalar.activation(e[:, :, co:DG],
                                         s_ps[:, :, co:DG],
                                         AF.Exp, scale=0.125)
                    # causal mask on the diagonal 128-col block
                    if P * j >= DG * n:
                        nc.vector.tensor_mul(e[:, :, co:co + P],
                                             e[:, :, co:co + P],
                                             tri2[:])
                    es[(n, j)] = e

                def emit_ctx(n, j):
                    co = max(0, P * j - DG * n)
                    e = es.pop((n, j))
                    st, sp = (j == 0), (j == jmax[n] - 1)
                    for h in range(2):
                        hh = 2 * p + h
                        nc.tensor.matmul(
                            cxp[n][:, h, co:DG],
                            v1[:, j, VS * hh:VS * hh + VS],
                            e[:, h, co:DG], start=st, stop=sp,
                            skip_group_check=True)

                def make_norm(n, npool=None, ntag0="s", ntag1="s"):
                    npool = npool or sc
                    # normalize, split into small steps sprinkled by the
                    # piece loop (a DVE burst at a pair boundary stalls
                    # the PE and can re-throttle the HAM clock gate).
                    # PE-broadcast the raw f16-staged denominator to 64
                    # rows at base 0, reciprocal there (custom-DVE recip
                    # is base-0 only), multiply; h1's normalized ctx
                    # moves to partitions 64:128 via an identity matmul.
                    # All DVE ops partition-aligned.
                    cxn = cxp[n]
                    qsl = slice(DG * n, DG * (n + 1))
                    st = {}

                    def s0():
                        st["ddf"] = npl.tile([P, 2, DG], f16, tag="ddf",
                                             name="ddf")
                        nc.vector.tensor_copy(st["ddf"][64:65, :, :],
                                              cxn[64:65, :, :])

                    def s1():
                        st["rbd"] = rbd = npool.tile([P, 2, DG], f32,
                                                     tag=ntag0,
                                                     name="rbd")
                        for h in range(2):
                            nc.tensor.matmul(rbd[0:64, h, :],
                                             ones[64:65, 0:64],
                                             st["ddf"][64:65, h, :],
                                             start=True, stop=True,
                                             tile_position=(64, 0),
                                             skip_group_check=True)

                    def s2():
                        st["rbs"] = rbs = npl.tile([64, 2, DG], f32,
                                                   tag="rbs", name="rbs")
                        nc.vector.reciprocal_approx_fast(
                            rbs[:], st["rbd"][0:64, :, :])

                    def s3():
                        nc.vector.tensor_mul(cn[0:64, p, qsl],
                                             cxn[0:64, 0, :],
                                             st["rbs"][:, 0, :])

                    def s4():
                        st["h1n"] = h1n = npl.tile([64, DG], f16,
                                                   tag="h1n", name="h1n")
                        nc.vector.tensor_mul(h1n[:], cxn[0:64, 1, :],
                                             st["rbs"][:, 1, :])

                    def s5():
                        st["pmv"] = pmv = npool.tile([P, DG], f32,
                                                     tag=ntag1,
                                                     name="pmv")
                        nc.tensor.matmul(pmv[64:P, :], i64[:],
                                         st["h1n"][:],
                                         start=True, stop=True,
                                         tile_position=(0, 64),
                                         skip_group_check=True)

                    def s6():
                        nc.vector.tensor_copy(cn[64:P, p, qsl],
                                              st["pmv"][64:P, :])

                    return [s0, s1, s2, s3, s4, s5, s6]

                slots = ([(0, j) for j in range(4)]
                         + [(1, j) for j in range(8)])
                LAG = 3
                for i in range(len(slots) + LAG):
                    if i < len(slots):
                        emit_scores(*slots[i])
                    if norm_pend:
                        norm_pend.pop(0)()
                    if i >= LAG:
                        n, j = slots[i - LAG]
                        emit_ctx(n, j)
                        if (n, j) == (0, jmax[0] - 1):
                            norm_pend.extend(make_norm(0))
                if p == TT - 1:
                    norm_pend.extend(make_norm(1, cx, "cx0", "cx1"))
                else:
                    norm_pend.extend(make_norm(1))

            # phase A: all projections. K via k-major groups (starts as
            # soon as the first DMAs land), Q with the rope software
            # pipeline, V with ScalarE evac.
            with tc.tile_pool(name="ppA", bufs=8, space="PSUM") as ppA:
                g1 = proj_k_group(ppA, (0, 1, 2, 3))
                for ps, dsts, bias, csl in g1:
                    rope_evac(ps, dsts, bias, csl)
                for t in range(TT):
                    proj_q(ppA, t)
                while rope_pend:
                    rope_finish(ppA)
                for i in range(NT):
                    proj_v(ppA, i)

            # phase B: attention (softmax exp on ScalarE overlaps the
            # PE scores/ctx stream; normalize is half-pipelined)
            with tc.tile_pool(name="cx", bufs=1, space="PSUM") as cx:
                with tc.tile_pool(name="sc", bufs=2,
                                  space="PSUM") as sc:
                    for p in range(TT):
                        attn_pair(sc, cx, p)

            if taps:
                for tn, tile_ap in (("qz", qz), ("kt", kt), ("v1", v1),
                                    ("cn", cn)):
                    nc.sync.dma_start(tap_ext[tn][:], tile_ap[:])

            # ---- partial output projection (natural [s, dout]) ----
                with tc.tile_pool(name="op", bufs=4,
                                  space="PSUM") as op, \
                     tc.tile_pool(name="ob", bufs=4) as ob:
                    # bo is added host-side after summing the partials;
                    # the last pair's normalize steps (cx-pool psum)
                    # interleave with the first chunks here
                    for i in range(NT):
                        ssl = slice(P * i, P * (i + 1))
                        for c in range(2):
                            csl = slice(DG * c, DG * (c + 1))
                            yp = op.tile([P, DG], f32, tag="yp",
                                         name="yp")
                            for t in range(TT):
                                nc.tensor.matmul(yp[:], cn[:, t, ssl],
                                                 wo[:, t, csl],
                                                 start=(t == 0),
                                                 stop=(t == TT - 1))
                            if norm_pend:
                                norm_pend.pop(0)()
                            ys = ob.tile([P, DG], f16, tag="ys",
                                         name="ys")
                            if (2 * i + c) % 2 == 0:
                                nc.scalar.activation(ys[:], yp[:],
                                                     AF.Copy)
                            else:
                                nc.vector.tensor_copy(ys[:], yp[:])
                            q = (nc.sync, nc.gpsimd)[(2 * i + c) % 2]
                            q.dma_start(y_sh[ssl, csl], ys[:])

    nc.compile()
    return nc


def _host_tables():
    # RoPE tables, computed in float32 to match the reference's jnp path.
    pos = np.arange(S, dtype=np.float32)
    inv = np.exp(np.arange(0, Dh, 2, dtype=np.float32)
                 * np.float32(-np.log(10000.0) / Dh))          # [32]
    ang = pos[:, None] * inv[None, :]                          # [S, 32]
    sin = np.sin(ang).astype(np.float32)
    cos = np.cos(ang).astype(np.float32)
    # per-partition pattern for [2 heads x 64, s] transposed layout
    dd = np.arange(P) % Dh
    cosP = np.empty((P, S), np.float32)
    sinP = np.empty((P, S), np.float32)
    lo = dd < 32
    cosP[lo] = cos[:, dd[lo]].T
    sinP[lo] = -sin[:, dd[lo]].T
    cosP[~lo] = cos[:, dd[~lo] - 32].T
    sinP[~lo] = sin[:, dd[~lo] - 32].T
    return cosP.astype(np.float16), sinP.astype(np.float16)


def _perm128():
    p = np.zeros((P, P), np.float16)
    i = np.arange(P)
    p[i, i ^ 32] = np.float16(1.0)
    return p


def _tile_T(a):
    # [rows, D] -> [NT, P, rows]: k-tile-major transpose (contiguous DMA)
    rows = a.shape[0]
    return np.ascontiguousarray(a.T.reshape(NT, P, rows))


def make_in_maps(x, Wq, bq, Wk, bk, Wv, bv, Wo, bo):
    x = np.asarray(x, np.float16)
    Wq = np.asarray(Wq, np.float16)
    Wk = np.asarray(Wk, np.float16)
    Wv = np.asarray(Wv, np.float16)
    Wo = np.asarray(Wo, np.float16)
    bq = np.asarray(bq, np.float16).astype(np.float32)
    bk = np.asarray(bk, np.float16).astype(np.float32)
    cosP, sinP = _host_tables()
    r = np.arange(P)[:, None]
    c = np.arange(P)[None, :]
    tri = (c >= r).astype(np.float16)                     # [kv, q] valid
    tri2 = np.ascontiguousarray(
        np.broadcast_to(tri[:, None, :], (P, 2, P)))
    shared = {
        "cosk": cosP, "sink": sinP, "tri2": tri2, "p128": _perm128(),
        "i64": np.eye(64, dtype=np.float16),
    }
    xt_by_batch = [_tile_T(x[b]) for b in range(B)]

    in_maps = []
    for core in range(N_CORES):
        b, g = core // 2, core % 2
        gsl = slice(DG * g, DG * (g + 1))
        m = {
            "xt_sh": xt_by_batch[b],
            "wq": np.ascontiguousarray(Wq[:, gsl]),
            "wk": np.ascontiguousarray(Wk[:, gsl]),
            "wv": np.ascontiguousarray(Wv[:, gsl]),
            "wo": np.ascontiguousarray(Wo[gsl, :]),
            "bqt": np.ascontiguousarray(bq[gsl].reshape(TT, P).T),
            "bkt": np.ascontiguousarray(bk[gsl].reshape(TT, P).T),
            "bv": np.asarray(bv, np.float16)[gsl].reshape(1, DG),
        }
        m.update(shared)
        in_maps.append(m)
    return in_maps


def kernel(x, Wq, bq, Wk, bk, Wv, bv, Wo, bo):
    from concourse.bass_utils import run_bass_kernel_spmd

    with _lock:
        if "nc" not in _cache:
            _cache["nc"] = _build_program()
    nc = _cache["nc"]

    in_maps = make_in_maps(x, Wq, bq, Wk, bk, Wv, bv, Wo, bo)
    res = run_bass_kernel_spmd(nc, in_maps, list(range(N_CORES)))

    out = np.empty((B, S, D), np.float16)
    bo_f = np.asarray(bo, np.float32).reshape(1, D)
    for b in range(B):
        y0 = res.results[2 * b]["y_sh"].astype(np.float32)
        y1 = res.results[2 * b + 1]["y_sh"].astype(np.float32)
        out[b] = (y0 + y1 + bo_f).astype(np.float16)
    return out
